# revision 1
# baseline (speedup 1.0000x reference)
"""PoolNet (social-GAN pooling) Trainium2 kernel.

Math (reference semantics, eval-mode BN):
  h1[f,i,j] = relu(bn1(concat(emb(pos_j - pos_i), h_j) @ W1 + b1))
  h2[f,i,j] = relu(bn2(h1 @ W2 + b2))
  out[f,i]  = max_j h2[f,i,j]

Two algebraic reductions let us skip most of the work:
  1. Layer 1 collapses: emb@W1e = (pos_j - pos_i)@(We@W1e) + be@W1e, so
       bn1(x@W1+b1) = u[f,j] - v[f,i]
     with u = pos@A' + h@W1h' + c1 and v = pos@A' (A', W1h', c1 are
     host-folded weights including the BN1 affine).  The (F,P,P,192)
     concat tensor never exists.
  2. relu/bias are monotone, so max_j relu(z_j + c2) = relu(max_j z_j + c2):
     the 64-way max pool runs on the raw matmul accumulator and the final
     ReLU touches only the pooled (B, D) tensor.

Sharding: data-parallel over frames, 4 frames per core on 8 cores, no
cross-core communication.  Matmuls run in float32r (FP22 multiplies,
full-speed PE path).
"""

import sys

for _p in ("/opt/trn_rl_repo",):
    if _p not in sys.path:
        sys.path.insert(0, _p)

from contextlib import ExitStack

import numpy as np

import concourse.bass as bass
import concourse.mybir as mybir
import concourse.tile as tile
from concourse import bacc
from concourse.bass_utils import run_bass_kernel_spmd
from concourse.masks import make_identity

EPS = 1e-5
F, P, B, H, E, M, D = 32, 64, 2048, 128, 64, 512, 1024
NCORES = 8
FC = F // NCORES  # frames per core
RPC = FC * P  # (frame, ped) rows per core = 256
QK = M // 128  # layer-2 contraction chunks = 4
QM = D // 128  # layer-2 output chunks = 8
IH_DEFAULT = 32  # i-rows per half-frame block
HB_DEFAULT = IH_DEFAULT * P

_CACHE = {}


def _build_nc(
    loop_iters=1,
    w2_bf16=False,
    h1_bf16=False,
    n_tile=512,
    h1_bufs=12,
    raw_bufs=4,
    ps_cols=1024,
    ps_bufs=4,
    tail_spread=True,
    ih=16,
):
    IH = ih
    HB = IH * P
    f32 = mybir.dt.float32
    f32r = mybir.dt.float32r
    bf16 = mybir.dt.bfloat16
    w2_dt = bf16 if w2_bf16 else f32r
    h1_dt = bf16 if h1_bf16 else f32r
    AF = mybir.ActivationFunctionType

    nc = bacc.Bacc("TRN2", target_bir_lowering=False, debug=False)

    pos_t = nc.dram_tensor("pos_t", [2, RPC], f32r, kind="ExternalInput").ap()
    h_t = nc.dram_tensor("h_t", [H, RPC], f32r, kind="ExternalInput").ap()
    w2 = nc.dram_tensor("w2", [M, D], w2_dt, kind="ExternalInput").ap()
    w1h = nc.dram_tensor("w1h", [H, M], f32r, kind="ExternalInput").ap()
    a2 = nc.dram_tensor("a2", [2, M], f32r, kind="ExternalInput").ap()
    c1c = nc.dram_tensor("c1c", [128, QK], f32, kind="ExternalInput").ap()
    c2c = nc.dram_tensor("c2c", [128, QM], f32, kind="ExternalInput").ap()
    out = nc.dram_tensor("out", [RPC, D], f32, kind="ExternalOutput").ap()

    with ExitStack() as ctx:
        tc = ctx.enter_context(tile.TileContext(nc))
        consts = ctx.enter_context(tc.tile_pool(name="consts", bufs=1))
        data = ctx.enter_context(tc.tile_pool(name="data", bufs=1))

        w2sb = consts.tile([128, QK, D], w2_dt)
        w2r = w2.rearrange("(k p) m -> p k m", p=128)
        for k in range(QK):
            nc.sync.dma_start(out=w2sb[:, k], in_=w2r[:, k])
        w1hsb = consts.tile([H, M], f32r)
        nc.sync.dma_start(out=w1hsb, in_=w1h)
        a2sb = consts.tile([2, M], f32r)
        nc.sync.dma_start(out=a2sb, in_=a2)
        c1sb = consts.tile([128, QK], f32)
        nc.sync.dma_start(out=c1sb, in_=c1c)
        c2sb = consts.tile([128, QM], f32)
        nc.sync.dma_start(out=c2sb, in_=c2c)
        possb = consts.tile([2, RPC], f32r)
        nc.sync.dma_start(out=possb, in_=pos_t)
        htsb = consts.tile([H, RPC], f32r)
        nc.sync.dma_start(out=htsb, in_=h_t)
        ident = consts.tile([128, 128], f32)
        make_identity(nc, ident)

        u_sb = data.tile([128, QK, RPC], f32)
        v_sb = data.tile([128, QK, RPC], f32)
        pool_sb = data.tile([128, QM, RPC], f32)
        out_sb = data.tile([128, 2, D], f32)

        h1raw = ctx.enter_context(tc.tile_pool(name="h1raw", bufs=raw_bufs))
        h1p = ctx.enter_context(tc.tile_pool(name="h1", bufs=h1_bufs))
        tmp = ctx.enter_context(tc.tile_pool(name="tmp", bufs=2))
        pspool = ctx.enter_context(tc.tile_pool(name="ps", bufs=ps_bufs, space="PSUM"))

        out_r = out.rearrange("(h p) c -> p h c", p=128)

        def body():
            # u = pos@A' + h@W1h' + c1, v = pos@A', channels-on-partition.
            for q in range(QK):
                ms = slice(q * 128, (q + 1) * 128)
                psu = pspool.tile([128, ps_cols], f32, tag="ps")
                nc.tensor.matmul(
                    psu[:, :RPC], lhsT=w1hsb[:, ms], rhs=htsb, start=True, stop=False
                )
                nc.tensor.matmul(
                    psu[:, :RPC], lhsT=a2sb[:, ms], rhs=possb, start=False, stop=True
                )
                nc.scalar.activation(
                    u_sb[:, q],
                    psu[:, :RPC],
                    AF.Identity,
                    bias=c1sb[:, q : q + 1],
                    scale=1.0,
                )
                psv = pspool.tile([128, ps_cols], f32, tag="ps")
                nc.tensor.matmul(
                    psv[:, :RPC], lhsT=a2sb[:, ms], rhs=possb, start=True, stop=True
                )
                nc.scalar.copy(v_sb[:, q], psv[:, :RPC])

            def emit_tail(half):
                # relu(pool + c2) for 128 pooled rows, transpose, stage for the
                # output DMA.  Runs as soon as those rows' reduces are done.
                for m in range(QM):
                    pb = tmp.tile([128, 128], f32, tag="pb")
                    nc.scalar.activation(
                        pb,
                        pool_sb[:, m, half * 128 : (half + 1) * 128],
                        AF.Relu,
                        bias=c2sb[:, m : m + 1],
                        scale=1.0,
                    )
                    pst = pspool.tile([128, ps_cols], f32, tag="ps")
                    nc.tensor.transpose(pst[:, :128], pb, ident)
                    nc.scalar.copy(
                        out_sb[:, half, m * 128 : (m + 1) * 128], pst[:, :128]
                    )
                nc.sync.dma_start(
                    out=out_r[:, half], in_=out_sb[:, half]
                )

            def make_h1(blk):
                # h1 = relu(u_j - v_i) for one half-frame block of 2048 rows.
                i0 = blk * IH
                f = i0 // P
                h1 = []
                for q in range(QK):
                    raw = h1raw.tile([128, IH, P], f32, tag="raw")
                    u_b = (
                        u_sb[:, q, f * P : (f + 1) * P]
                        .unsqueeze(1)
                        .broadcast_to((128, IH, P))
                    )
                    v_b = (
                        v_sb[:, q, i0 : i0 + IH]
                        .unsqueeze(2)
                        .broadcast_to((128, IH, P))
                    )
                    nc.vector.tensor_sub(raw, u_b, v_b)
                    t = h1p.tile([128, HB], h1_dt, tag="h1")
                    nc.scalar.activation(t, raw.rearrange("p a b -> p (a b)"), AF.Relu)
                    h1.append(t)
                return h1

            NBLK = (FC * P) // IH
            for blk in range(NBLK):
                i0 = blk * IH
                f = i0 // P
                h1 = make_h1(blk)
                n_ps = HB // ps_cols
                ips = ps_cols // P  # i-rows pooled per psum tile
                for m in range(QM):
                    for ip in range(n_ps):
                        ps = pspool.tile([128, ps_cols], f32, tag="ps")
                        for nt in range(ps_cols // n_tile):
                            base = ip * ps_cols + nt * n_tile
                            ns = slice(base, base + n_tile)
                            for k in range(QK):
                                nc.tensor.matmul(
                                    ps[:, nt * n_tile : (nt + 1) * n_tile],
                                    lhsT=w2sb[:, k, m * 128 : (m + 1) * 128],
                                    rhs=h1[k][:, ns],
                                    start=(k == 0),
                                    stop=(k == QK - 1),
                                )
                        nc.vector.reduce_max(
                            pool_sb[:, m, i0 + ip * ips : i0 + (ip + 1) * ips],
                            ps.rearrange("p (a b) -> p a b", b=P),
                            axis=mybir.AxisListType.X,
                        )
                if tail_spread and (blk + 1) * IH % 128 == 0:
                    emit_tail(((blk + 1) * IH) // 128 - 1)
            if not tail_spread:
                emit_tail(0)
                emit_tail(1)

        if loop_iters == 1:
            body()
        else:
            with tc.For_i(0, loop_iters, 1):
                body()

    nc.compile()
    return nc


def _fold_weights(We, be, W1, b1, g1, beta1, W2, b2, g2, beta2, rm1, rv1, rm2, rv2):
    f8 = np.float64
    We, be, W1, b1 = We.astype(f8), be.astype(f8), W1.astype(f8), b1.astype(f8)
    g1, beta1, rm1, rv1 = (
        g1.astype(f8),
        beta1.astype(f8),
        rm1.astype(f8),
        rv1.astype(f8),
    )
    W2, b2, g2, beta2, rm2, rv2 = (
        W2.astype(f8),
        b2.astype(f8),
        g2.astype(f8),
        beta2.astype(f8),
        rm2.astype(f8),
        rv2.astype(f8),
    )
    s1 = g1 / np.sqrt(rv1 + EPS)
    W1e = W1[:E]
    Ap = (We @ W1e) * s1  # (2, M)
    W1hp = W1[E:] * s1  # (H, M)
    c1 = s1 * (be @ W1e + b1 - rm1) + beta1  # (M,)
    s2 = g2 / np.sqrt(rv2 + EPS)
    W2p = W2 * s2  # (M, D)
    c2 = s2 * (b2 - rm2) + beta2  # (D,)
    asf = lambda x: np.ascontiguousarray(x, dtype=np.float32)
    return (
        asf(Ap),
        asf(W1hp),
        asf(c1.reshape(QK, 128).T),
        asf(W2p),
        asf(c2.reshape(QM, 128).T),
    )


def _prepare_in_maps(curr_h_states, curr_pos, w2_bf16=False, **weights):
    Ap, W1hp, c1c, W2p, c2c = _fold_weights(**weights)
    if w2_bf16:
        import ml_dtypes

        W2p = W2p.astype(ml_dtypes.bfloat16)
    h_full = np.asarray(curr_h_states, dtype=np.float32).reshape(B, H)
    pos_full = np.asarray(curr_pos, dtype=np.float32)
    in_maps = []
    for c in range(NCORES):
        r0, r1 = c * RPC, (c + 1) * RPC
        in_maps.append(
            {
                "pos_t": np.ascontiguousarray(pos_full[r0:r1].T),
                "h_t": np.ascontiguousarray(h_full[r0:r1].T),
                "w2": W2p,
                "w1h": W1hp,
                "a2": Ap,
                "c1c": c1c,
                "c2c": c2c,
            }
        )
    return in_maps


def _get_nc(loop_iters=1, **opts):
    key = ("nc", loop_iters, tuple(sorted(opts.items())))
    if key not in _CACHE:
        _CACHE[key] = _build_nc(loop_iters, **opts)
    return _CACHE[key]


def _make_in_maps(inputs, w2_bf16=False):
    return _prepare_in_maps(
        curr_h_states=inputs["curr_h_states"],
        curr_pos=inputs["curr_pos"],
        w2_bf16=w2_bf16,
        We=np.asarray(inputs["We"]),
        be=np.asarray(inputs["be"]),
        W1=np.asarray(inputs["W1"]),
        b1=np.asarray(inputs["b1"]),
        g1=np.asarray(inputs["g1"]),
        beta1=np.asarray(inputs["beta1"]),
        W2=np.asarray(inputs["W2"]),
        b2=np.asarray(inputs["b2"]),
        g2=np.asarray(inputs["g2"]),
        beta2=np.asarray(inputs["beta2"]),
        rm1=np.asarray(inputs["rm1"]),
        rv1=np.asarray(inputs["rv1"]),
        rm2=np.asarray(inputs["rm2"]),
        rv2=np.asarray(inputs["rv2"]),
    )


def run(inputs, trace=False, loop_iters=1, opts=None, **kw):
    """Build in_maps from full inputs, run on 8 cores, return BassKernelResults."""
    opts = opts or {}
    in_maps = _make_in_maps(inputs, w2_bf16=opts.get("w2_bf16", False))
    nc = _get_nc(loop_iters, **opts)
    return run_bass_kernel_spmd(
        nc, in_maps, core_ids=list(range(NCORES)), trace=trace, **kw
    )


def kernel(**inputs):
    res = run(inputs, trace=False)
    return np.concatenate([res.results[c]["out"] for c in range(NCORES)], axis=0)

